# revision 25
# baseline (speedup 1.0000x reference)
"""Trainium2 Bass kernel for batched multi-head attention with per-batch mask.

Reference semantics (B=8, H=16, S=1024, D=64):
    scores = Q @ K^T                      # [B,H,S,S]
    scores = where(mask[b], -1e9, scores) # per-batch mask
    attn   = softmax(scores / sqrt(D))
    out    = attn @ V

Key observations used here:
  * A masked batch (mask[b]=True) has constant scores, so softmax is exactly
    uniform (1/S) and out[b,h,q,:] = mean_k V[b,h,k,:].  That degenerate case
    is computed directly on host; only unmasked (b,h) units go to the device.
  * For unmasked units |scores/8| <= ~7, so softmax without max-subtraction
    (exp(x)/sum exp(x)) is numerically safe and mathematically identical.
  * Unmasked units are embarrassingly parallel; they are balanced across the
    8 NeuronCores (H=16 and B=8 make the unit count divisible by 8).

Device algorithm per unit (S=1024 split into 8 chunks of 128 along k):
  mm1:  T[k,q]  = sum_d K[k,d]*Q[q,d]       (lhsT = K^T chunk, rhs = Q^T)
        K=64 matmuls run at half rate, so chunk pairs are packed into row
        groups (0,0)/(64,0) with Q^T/K^T replicated on partitions 64..127 —
        the two half-array matmuls execute concurrently.
  exp:  E[k,q]  = exp(T/8)                  (ScalarE, PSUM->SBUF, fp32r out)
  mm2:  U[m,q] += sum_k Vx[k,m]*E[k,q]      (lhsT = Vx chunk = [V | ones | 0])
        Vx is zero-padded to 128 columns so M=128 stays on the fast path.
        -> U[0:64,q] = unnormalized out^T, U[64,q] = softmax denominator
  out:  reciprocal of the denominator row (bounced to a [32,32] layout so
        the slow DVE reciprocal sees a tiny free dim), GPSIMD partition
        broadcast, one DVE multiply, write out^T [64, S]; host transposes.
"""

import numpy as np

B, H, S, D = 8, 16, 1024, 64
P = 128                      # SBUF partitions / k-chunk size
NCHUNK = S // P              # 8 k-chunks per unit
NHALF = 2                    # matmul moving operand is limited to N=512 fp32
NCORES = 8

_program_cache = {}


def _build_program(n_units):
    import concourse.mybir as mybir
    import concourse.tile as tile
    from concourse import bacc

    f32 = mybir.dt.float32
    f32r = mybir.dt.float32r
    nc = bacc.Bacc("TRN2", target_bir_lowering=False, debug=False)

    # qt/kt carry Q^T/K^T duplicated on partitions 64..127 (row-group packing)
    qt_d = nc.dram_tensor("qt", [n_units, P, S], f32r, kind="ExternalInput").ap()
    kt_d = nc.dram_tensor("kt", [n_units, P, S], f32r, kind="ExternalInput").ap()
    vx_d = nc.dram_tensor("vx", [n_units, S, P], f32r, kind="ExternalInput").ap()
    out_d = nc.dram_tensor("out", [n_units, D, S], f32, kind="ExternalOutput").ap()

    with tile.TileContext(nc) as tc:
        with (
            tc.tile_pool(name="qp", bufs=2) as qp,
            tc.tile_pool(name="kp", bufs=2) as kp,
            tc.tile_pool(name="vp", bufs=2) as vp,
            tc.tile_pool(name="ep", bufs=4) as ep,
            tc.tile_pool(name="rp", bufs=2) as rp,
            tc.tile_pool(name="sp", bufs=2) as sp,
            tc.tile_pool(name="bp", bufs=2) as bp,
            tc.tile_pool(name="op", bufs=2) as op,
            tc.tile_pool(name="wp", bufs=1) as wpool,
            tc.tile_pool(name="pt", bufs=3, space="PSUM") as pt,   # 3 x 2 banks
            tc.tile_pool(name="pu", bufs=1, space="PSUM") as pu,   # 2 banks
        ):
            # PE warmup: ~4.5us of junk matmuls issued before the first real
            # matmul.  They run while the first unit's DMAs are in flight and
            # lift the PE HAM clock gate from 1.2 to 2.4 GHz, so real matmuls
            # start warm instead of paying a ~7us cold ramp.
            wk = wpool.tile([P, 512], mybir.dt.bfloat16)
            nc.vector.memset(wk, 0.0)
            w_ps = pt.tile([P, S], f32, tag="tps", name="warm")
            for i in range(9):
                nc.tensor.matmul(
                    w_ps[:, 0:512], lhsT=wk[:, 0:P], rhs=wk,
                    start=True, stop=True,
                )

            ones_f = wpool.tile([1, D], f32, name="ones_f")
            nc.vector.memset(ones_f, 1.0)
            ones_r = wpool.tile([1, D], f32r, name="ones_r")
            nc.vector.tensor_copy(out=ones_r, in_=ones_f)

            pending = None
            for j in range(n_units):
                # split the Q^T/K^T loads so the first matmul pair only waits
                # on half the data; subtile deps track per-slice readiness.
                qt = qp.tile([P, S], f32r)
                kt = kp.tile([P, S], f32r)
                nc.sync.dma_start(qt[:, 0:512], qt_d[j][:, 0:512])
                nc.sync.dma_start(kt[:, 0:512], kt_d[j][:, 0:512])
                nc.sync.dma_start(qt[:, 512:S], qt_d[j][:, 512:S])
                nc.sync.dma_start(kt[:, 512:S], kt_d[j][:, 512:S])
                vx = vp.tile([P, NCHUNK, P], f32r)
                nc.sync.dma_start(vx, vx_d[j].rearrange("(c p) d -> p c d", p=P))

                u_ps = pu.tile([P, S], f32)

                def mm1_pair(cp, qt=qt, kt=kt):
                    ca, cb = 2 * cp, 2 * cp + 1
                    ta = pt.tile([P, S], f32, tag="tps", name=f"ta{cp}")
                    tb = pt.tile([P, S], f32, tag="tps", name=f"tb{cp}")
                    for h in range(NHALF):
                        qs = slice(h * 512, (h + 1) * 512)
                        nc.tensor.matmul(
                            ta[:, qs],
                            lhsT=kt[0:D, ca * P:(ca + 1) * P],
                            rhs=qt[0:D, qs],
                            start=True, stop=True,
                            tile_position=(0, 0),
                        )
                        nc.tensor.matmul(
                            tb[:, qs],
                            lhsT=kt[D:P, cb * P:(cb + 1) * P],
                            rhs=qt[D:P, qs],
                            start=True, stop=True,
                            tile_position=(64, 0),
                        )
                    return ta, tb

                # software-pipelined chunk loop: the next pair's mm1s are
                # emitted (adjacently, for row-group packing) before this
                # pair's exp+mm2 consumers.
                NP = NCHUNK // 2
                tiles = mm1_pair(0)
                for cp in range(NP):
                    nxt = mm1_pair(cp + 1) if cp + 1 < NP else None
                    for c, t_ps in zip((2 * cp, 2 * cp + 1), tiles):
                        e_sb = ep.tile([P, S], f32r)
                        nc.scalar.activation(
                            e_sb, t_ps, mybir.ActivationFunctionType.Exp,
                            bias=0.0, scale=0.125,
                        )
                        for h in range(NHALF):
                            qs = slice(h * 512, (h + 1) * 512)
                            nc.tensor.matmul(
                                u_ps[:, qs],
                                lhsT=vx[:, c, :],
                                rhs=e_sb[:, qs],
                                start=(c == 0),
                                stop=(c == NCHUNK - 1),
                            )
                    tiles = nxt

                # U rows 0..63 hold out^T, row 64 holds the softmax
                # denominator.  Copy U out of PSUM immediately (releases
                # u_ps for the next unit).  The multiply for unit j-1 is
                # emitted here (its broadcast DMA finished a unit ago) so
                # the in-order DVE never blocks on this unit's DMA chain.
                # DVE reciprocal costs ~6.4ns per free-element regardless of
                # partition count, so bounce the 1024 denominators into a
                # [32, 32] layout first (free dim 32) and back.
                u_sb = rp.tile([D + 1, S], f32)
                nc.vector.tensor_copy(out=u_sb, in_=u_ps[0:D + 1, :])

                def finish(pend):
                    u_prev, rb_prev, j_prev = pend
                    o_sb = op.tile([D, S], f32)
                    for h in range(NHALF):
                        qs = slice(h * 512, (h + 1) * 512)
                        nc.vector.tensor_mul(
                            out=o_sb[:, qs], in0=u_prev[0:D, qs],
                            in1=rb_prev[0:D, qs],
                        )
                        nc.sync.dma_start(out_d[j_prev][:, qs], o_sb[:, qs])

                if pending is not None:
                    finish(pending)
                d_sm = sp.tile([32, 32], f32)
                nc.sync.dma_start(d_sm, u_sb[D:D + 1, :])
                r_sm = sp.tile([32, 32], f32, name="r_sm")
                nc.vector.reciprocal(out=r_sm, in_=d_sm)
                if j == n_units - 1:
                    # PE is idle after the last unit's matmuls: broadcast the
                    # reciprocal row with a rank-1 (K=1) matmul instead of
                    # the ~3.4us GPSIMD ucode path.
                    r_smr = sp.tile([32, 32], f32r, name="r_smr")
                    nc.vector.tensor_copy(out=r_smr, in_=r_sm)
                    r_row = sp.tile([1, S], f32r, name="r_rowr")
                    nc.sync.dma_start(r_row, r_smr)
                    r_bc = pt.tile([D, S], f32, tag="tps", name="bc")
                    for h in range(NHALF):
                        qs = slice(h * 512, (h + 1) * 512)
                        nc.tensor.matmul(
                            r_bc[0:D, qs], lhsT=ones_r, rhs=r_row[:, qs],
                            start=True, stop=True,
                        )
                else:
                    r_row = sp.tile([1, S], f32, name="r_row")
                    nc.sync.dma_start(r_row, r_sm)
                    r_bc = bp.tile([D, S], f32)
                    nc.gpsimd.partition_broadcast(r_bc, r_row)
                pending = (u_sb, r_bc, j)
            finish(pending)
    nc.compile()
    return nc


def _get_program(n_units):
    if n_units not in _program_cache:
        _program_cache[n_units] = _build_program(n_units)
    return _program_cache[n_units]


def _round_fp32r(x):
    """Round fp32 to the fp32r-representable set (bf16 hi + bf16 lo pair).

    The walrus verifier requires fp32r matmul operands to be pre-rounded;
    the PE's replicated fp32 path decomposes each value into two bf16s.
    """
    import ml_dtypes

    hi = x.astype(ml_dtypes.bfloat16).astype(np.float32)
    lo = (x - hi).astype(ml_dtypes.bfloat16).astype(np.float32)
    return hi + lo


def _prepare(Q, K, V, mask):
    """Host-side sharding. Returns (out_skeleton, units_per_core, in_maps)."""
    Q = np.ascontiguousarray(Q, dtype=np.float32)
    K = np.ascontiguousarray(K, dtype=np.float32)
    V = np.ascontiguousarray(V, dtype=np.float32)
    mask_b = np.asarray(mask).reshape(B).astype(bool)

    out = np.empty((B, H, S, D), dtype=np.float32)

    # Masked batches: softmax over a constant row is exactly uniform -> mean of V.
    for b in np.nonzero(mask_b)[0]:
        mv = V[b].mean(axis=1, dtype=np.float32)          # [H, D]
        out[b] = np.broadcast_to(mv[:, None, :], (H, S, D))

    units = [(b, h) for b in range(B) if not mask_b[b] for h in range(H)]
    if not units:
        return out, None, None

    # Pad to a multiple of NCORES with duplicates (identical redundant work).
    n_per = -(-len(units) // NCORES)
    padded = units + [units[0]] * (n_per * NCORES - len(units))
    per_core = [padded[i::NCORES] for i in range(NCORES)]

    QT = _round_fp32r(Q.transpose(0, 1, 3, 2))            # [B,H,D,S]
    KT = _round_fp32r(K.transpose(0, 1, 3, 2))
    Vr = _round_fp32r(V)

    in_maps = []
    for core_units in per_core:
        qt = np.empty((len(core_units), P, S), np.float32)
        kt = np.empty((len(core_units), P, S), np.float32)
        vx = np.zeros((len(core_units), S, P), np.float32)
        for s, (b, h) in enumerate(core_units):
            qt[s, 0:D] = QT[b, h]
            qt[s, D:P] = QT[b, h]
            kt[s, 0:D] = KT[b, h]
            kt[s, D:P] = KT[b, h]
            vx[s, :, 0:D] = Vr[b, h]
            vx[s, :, D:P] = 1.0
        in_maps.append({"qt": qt, "kt": kt, "vx": vx})
    return out, per_core, in_maps


def _run_device(n_units, in_maps, trace=False, trace_cores=None):
    from concourse import bass_utils

    nc = _get_program(n_units)
    return bass_utils.run_bass_kernel_spmd(
        nc,
        in_maps,
        list(range(NCORES)),
        trace=trace,
        trace_cores=trace_cores,
    )


def kernel(Q, K, V, mask, _trace=False, _result_box=None):
    out, per_core, in_maps = _prepare(Q, K, V, mask)
    if in_maps is None:
        return out
    res = _run_device(len(per_core[0]), in_maps, trace=_trace)
    if _result_box is not None:
        _result_box.append(res)
    for i, core_units in enumerate(per_core):
        core_out = res.results[i]["out"]                  # [n, D, S]
        for s, (b, h) in enumerate(core_units):
            out[b, h] = core_out[s].T
    return out
